# revision 10
# baseline (speedup 1.0000x reference)
"""Trainium2 Bass kernel for nn_Net_8650064134725 (moe_routing).

Data-parallel over 8 NeuronCores: each core gets 128 of 1024 samples.
Conv stack + gate run in float32r (TF32-class matmul precision, needed
because top-k gate selection is discontinuous); experts run in bf16.

Layouts (per core, per 16-sample chunk; partition dim first):
  S1    [128=(sub4 x 32rows: 27 used=(dy,dx,ic)), (b4, 32, 32)]  conv1 im2col (host-built)
  map1  [128=(sub4, oc32), (b4, 34, 34)]  padded    conv1 out / conv2 in
  map2  [128=(sub4, oc32), (b4, 18, 18)]  padded    pool2 out / conv3 in
  map3  [128=(sub2, oc64), (n'8, 18, 18)] padded    conv3 out / conv4 in
  map4  [128=(sub2, oc64), (n'8, 10, 10)] padded    pool4 out / conv5 in
  map5  [128=(oc128),      (n16, 10, 10)] padded    conv5 out / conv6 in
  f     [128=(c128),       (n128, pix16)]           features (f32r + bf16)
Block-diagonal / replicated weights make every conv matmul use the full
partition dim (sample-packed channels).
"""

import numpy as np
import ml_dtypes

import concourse.bass as bass
import concourse.tile as tile
from concourse import bacc, mybir
from concourse.bass_utils import run_bass_kernel_spmd
from concourse.masks import make_identity

EPS = 1e-5
N_CORES = 8
B = 1024
B_LOC = B // N_CORES          # 128
CH = 16                       # samples per chunk
NCH = B_LOC // CH             # 8 chunks
SHIFTS = [(dy, dx) for dy in range(3) for dx in range(3)]

F32 = mybir.dt.float32
F32R = mybir.dt.float32r
BF16 = mybir.dt.bfloat16

LAST_RESULTS = None
LAST_NC = None
LAST_IN_MAPS = None
DEBUG = False


def _fold_bn(b, gamma, beta, mean, var):
    s = gamma / np.sqrt(var + EPS)
    t = (b - mean) * s + beta
    return s.astype(np.float32), t.astype(np.float32)


def _prep_host(x, conv_params, expert_params, gate_params):
    """Host-side layout prep. Returns (shared weight arrays, per-core xs1)."""
    x = np.asarray(x, np.float32)
    cp = [[np.asarray(a, np.float32) for a in p] for p in conv_params]
    ep = {k: np.asarray(v, np.float32) for k, v in expert_params.items()}
    gp = {k: np.asarray(v, np.float32) for k, v in gate_params.items()}

    shared = {}
    # conv weights
    W1 = cp[0][0]  # [32, 3, 3, 3] (oc, ic, dy, dx)
    w1r = W1.transpose(2, 3, 1, 0).reshape(27, 32)  # [(dy,dx,ic), oc]
    wc1 = np.zeros((128, 128), np.float32)
    for s in range(4):
        wc1[32 * s:32 * s + 27, 32 * s:32 * s + 32] = w1r
    shared["wc1"] = wc1

    def shift_blocks(W, reps, kron):
        # W [oc, ic, 3, 3] -> [9, 128, oc*?]
        out = []
        for dy, dx in SHIFTS:
            blk = W[:, :, dy, dx].T.astype(np.float32)  # [ic, oc]
            if kron:
                out.append(np.kron(np.eye(reps, dtype=np.float32), blk))
            else:
                out.append(np.tile(blk, (reps, 1)))
        return np.stack(out)

    shared["wc2"] = shift_blocks(cp[1][0], 4, kron=True)    # [9,128,128]
    # conv3: pass p maps input sub s4=2p+s2 -> output (s2, oc64)
    wc3 = np.zeros((2, 9, 128, 128), np.float32)
    for i, (dy, dx) in enumerate(SHIFTS):
        blk = cp[2][0][:, :, dy, dx].T.astype(np.float32)  # [ic32, oc64]
        for p in range(2):
            for s2 in range(2):
                s4 = 2 * p + s2
                wc3[p, i, 32 * s4:32 * s4 + 32, 64 * s2:64 * s2 + 64] = blk
    shared["wc3"] = wc3
    shared["wc4"] = shift_blocks(cp[3][0], 2, kron=True)    # [9,128,128]
    # conv5: pass p reads input rows 64p (sub p), writes all 128 oc
    wc5 = np.zeros((2, 9, 128, 128), np.float32)
    for i, (dy, dx) in enumerate(SHIFTS):
        blk = cp[4][0][:, :, dy, dx].T.astype(np.float32)  # [ic64, oc128]
        for p in range(2):
            wc5[p, i, 64 * p:64 * p + 64, :] = blk
    shared["wc5"] = wc5
    shared["wc6"] = shift_blocks(cp[5][0], 1, kron=False)   # [9,128,128]

    # bn scale/shift per layer, tiled to the psum partition layout
    reps = [4, 4, 2, 2, 1, 1]
    bn = np.zeros((128, 12), np.float32)
    for l in range(6):
        s, t = _fold_bn(*cp[l][1:])
        bn[:, l] = np.tile(s, reps[l])
        bn[:, 6 + l] = np.tile(t, reps[l])
    shared["bn"] = bn

    # gate
    gs, gt = _fold_bn(gp["b1"], gp["g"], gp["be"], gp["m"], gp["v"])
    shared["gst"] = np.stack([gs, gt], axis=1).astype(np.float32)  # [128, 2]
    shared["wg1"] = gp["w1"].reshape(128, 16, 128).transpose(1, 0, 2).copy()  # [16,128,128]
    shared["wg2"] = gp["w2"].astype(np.float32)  # [128, 8]
    shared["gb2"] = gp["b2"].reshape(8, 1).astype(np.float32)

    # experts (bf16)
    bf = ml_dtypes.bfloat16
    shared["we1"] = ep["w1"].reshape(8, 128, 16, 256).transpose(0, 2, 1, 3).astype(bf).copy()
    shared["we2"] = ep["w2"].reshape(8, 2, 128, 256).astype(bf).copy()
    shared["we3"] = ep["w3"].reshape(8, 2, 128, 10).astype(bf).copy()
    s1, t1 = _fold_bn(ep["b1"], ep["g1"], ep["be1"], ep["m1"], ep["v1"])  # [8,256]
    s2, t2 = _fold_bn(ep["b2"], ep["g2"], ep["be2"], ep["m2"], ep["v2"])
    shared["es1"] = s1.reshape(8, 2, 128).transpose(2, 0, 1).reshape(128, 16).copy()
    shared["et1"] = t1.reshape(8, 2, 128).transpose(2, 0, 1).reshape(128, 16).copy()
    shared["es2"] = s2.reshape(8, 2, 128).transpose(2, 0, 1).reshape(128, 16).copy()
    shared["et2"] = t2.reshape(8, 2, 128).transpose(2, 0, 1).reshape(128, 16).copy()
    shared["eb3"] = ep["b3"].T.astype(np.float32).copy()  # [10, 8]

    # per-core conv1 im2col input
    xs1_cores = []
    xpad = np.pad(x, ((0, 0), (0, 0), (1, 1), (1, 1)))
    win = np.lib.stride_tricks.sliding_window_view(xpad, (32, 32), axis=(2, 3))
    # win[n, ic, dy, dx, y, x] = xpad[n, ic, dy+y, dx+x]
    for k in range(N_CORES):
        v = win[k * B_LOC:(k + 1) * B_LOC]  # [128, 3, 3, 3, 32, 32]
        v = v.reshape(NCH, 4, 4, 3, 3, 3, 32, 32)  # [j, b, s, ic, dy, dx, y, x]
        v = v.transpose(0, 2, 4, 5, 3, 1, 6, 7)    # [j, s, dy, dx, ic, b, y, x]
        v = v.reshape(NCH, 4, 27, 4, 32, 32)
        xs1 = np.zeros((NCH, 4, 32, 4, 32, 32), np.float32)
        xs1[:, :, :27] = v
        xs1_cores.append(np.ascontiguousarray(xs1.reshape(NCH, 128, 4096)))
    return shared, xs1_cores


def _build(nc):
    """Emit the kernel IR. Returns nothing; tensors are declared on nc."""
    xs1_d = nc.dram_tensor("xs1", [NCH, 128, 4096], F32R, kind="ExternalInput")
    wc1_d = nc.dram_tensor("wc1", [128, 128], F32R, kind="ExternalInput")
    wc2_d = nc.dram_tensor("wc2", [9, 128, 128], F32R, kind="ExternalInput")
    wc3_d = nc.dram_tensor("wc3", [2, 9, 128, 128], F32R, kind="ExternalInput")
    wc4_d = nc.dram_tensor("wc4", [9, 128, 128], F32R, kind="ExternalInput")
    wc5_d = nc.dram_tensor("wc5", [2, 9, 128, 128], F32R, kind="ExternalInput")
    wc6_d = nc.dram_tensor("wc6", [9, 128, 128], F32R, kind="ExternalInput")
    bn_d = nc.dram_tensor("bn", [128, 12], F32, kind="ExternalInput")
    gst_d = nc.dram_tensor("gst", [128, 2], F32, kind="ExternalInput")
    wg1_d = nc.dram_tensor("wg1", [16, 128, 128], F32R, kind="ExternalInput")
    wg2_d = nc.dram_tensor("wg2", [128, 8], F32R, kind="ExternalInput")
    gb2_d = nc.dram_tensor("gb2", [8, 1], F32, kind="ExternalInput")
    we1_d = nc.dram_tensor("we1", [8, 16, 128, 256], BF16, kind="ExternalInput")
    we2_d = nc.dram_tensor("we2", [8, 2, 128, 256], BF16, kind="ExternalInput")
    we3_d = nc.dram_tensor("we3", [8, 2, 128, 10], BF16, kind="ExternalInput")
    es1_d = nc.dram_tensor("es1", [128, 16], F32, kind="ExternalInput")
    et1_d = nc.dram_tensor("et1", [128, 16], F32, kind="ExternalInput")
    es2_d = nc.dram_tensor("es2", [128, 16], F32, kind="ExternalInput")
    et2_d = nc.dram_tensor("et2", [128, 16], F32, kind="ExternalInput")
    eb3_d = nc.dram_tensor("eb3", [10, 8], F32, kind="ExternalInput")
    out_d = nc.dram_tensor("out", [128, 10], F32, kind="ExternalOutput")
    dbg = {}
    if DEBUG:
        dbg["map1"] = nc.dram_tensor("dbg_map1", [128, 4, 34, 34], F32R, kind="ExternalOutput")
        dbg["map2"] = nc.dram_tensor("dbg_map2", [128, 4, 18, 18], F32R, kind="ExternalOutput")
        dbg["map3"] = nc.dram_tensor("dbg_map3", [128, 8, 18, 18], F32R, kind="ExternalOutput")
        dbg["map4"] = nc.dram_tensor("dbg_map4", [128, 8, 10, 10], F32R, kind="ExternalOutput")
        dbg["map5"] = nc.dram_tensor("dbg_map5", [128, 16, 10, 10], F32R, kind="ExternalOutput")
        dbg["f"] = nc.dram_tensor("dbg_f", [128, 128, 16], F32R, kind="ExternalOutput")
        dbg["logits"] = nc.dram_tensor("dbg_logits", [8, 128], F32, kind="ExternalOutput")
        dbg["gates"] = nc.dram_tensor("dbg_gates", [128, 8], F32, kind="ExternalOutput")
        dbg["eo0"] = nc.dram_tensor("dbg_eo0", [10, 128], F32, kind="ExternalOutput")

    Relu = mybir.ActivationFunctionType.Relu
    Ident = mybir.ActivationFunctionType.Identity
    Exp = mybir.ActivationFunctionType.Exp
    OP = mybir.AluOpType

    with tile.TileContext(nc) as tc:
        wpool = tc.alloc_tile_pool(name="wpool", bufs=1)
        mpool = tc.alloc_tile_pool(name="mpool", bufs=2)
        tpool = tc.alloc_tile_pool(name="tpool", bufs=2)
        fpool = tc.alloc_tile_pool(name="fpool", bufs=1)
        m5pool = tc.alloc_tile_pool(name="m5pool", bufs=1)
        epool = tc.alloc_tile_pool(name="epool", bufs=12)
        hpool = tc.alloc_tile_pool(name="hpool", bufs=6)
        spool = tc.alloc_tile_pool(name="spool", bufs=1)
        cps = tc.alloc_tile_pool(name="cps", bufs=4, space="PSUM")
        eps = tc.alloc_tile_pool(name="eps", bufs=2, space="PSUM")
        tps = tc.alloc_tile_pool(name="tps", bufs=2, space="PSUM")

        # ---- resident weights ----
        wc1 = wpool.tile([128, 128], F32R, tag="wc1")
        nc.sync.dma_start(out=wc1, in_=wc1_d[:, :])
        wc = {}
        for name, dram in (("wc2", wc2_d), ("wc4", wc4_d), ("wc6", wc6_d)):
            for i in range(9):
                t = wpool.tile([128, 128], F32R, tag=f"{name}_{i}")
                nc.sync.dma_start(out=t, in_=dram[i, :, :])
                wc[(name, i)] = t
        for name, dram in (("wc3", wc3_d), ("wc5", wc5_d)):
            for p in range(2):
                for i in range(9):
                    t = wpool.tile([128, 128], F32R, tag=f"{name}_{p}_{i}")
                    nc.sync.dma_start(out=t, in_=dram[p, i, :, :])
                    wc[(name, p, i)] = t
        bnsb = wpool.tile([128, 12], F32, tag="bn")
        nc.sync.dma_start(out=bnsb, in_=bn_d[:, :])
        gst = wpool.tile([128, 2], F32, tag="gst")
        nc.sync.dma_start(out=gst, in_=gst_d[:, :])
        wg1 = []
        for p in range(16):
            t = wpool.tile([128, 128], F32R, tag=f"wg1_{p}")
            nc.sync.dma_start(out=t, in_=wg1_d[p, :, :])
            wg1.append(t)
        wg2 = wpool.tile([128, 8], F32R, tag="wg2")
        nc.sync.dma_start(out=wg2, in_=wg2_d[:, :])
        gb2 = wpool.tile([8, 1], F32, tag="gb2")
        nc.sync.dma_start(out=gb2, in_=gb2_d[:, :])
        es1 = wpool.tile([128, 16], F32, tag="es1")
        nc.sync.dma_start(out=es1, in_=es1_d[:, :])
        et1 = wpool.tile([128, 16], F32, tag="et1")
        nc.sync.dma_start(out=et1, in_=et1_d[:, :])
        es2 = wpool.tile([128, 16], F32, tag="es2")
        nc.sync.dma_start(out=es2, in_=es2_d[:, :])
        et2 = wpool.tile([128, 16], F32, tag="et2")
        nc.sync.dma_start(out=et2, in_=et2_d[:, :])
        eb3 = wpool.tile([10, 8], F32, tag="eb3")
        nc.sync.dma_start(out=eb3, in_=eb3_d[:, :])
        id8 = wpool.tile([8, 8], F32, tag="id8")
        make_identity(nc, id8[:, :])
        id10 = wpool.tile([10, 10], F32, tag="id10")
        make_identity(nc, id10[:, :])
        zline = wpool.tile([128, 160], F32, tag="zline")
        nc.vector.memset(zline, 0.0)

        f_r = fpool.tile([128, 128, 16], F32R, tag="f_r")
        f_b = fpool.tile([128, 128, 16], BF16, tag="f_b")

        def bscale(l, lo=0, n=128):
            return bnsb[lo:lo + n, l:l + 1], bnsb[lo:lo + n, 6 + l:7 + l]

        # ================= conv stack, per chunk =================
        for j in range(NCH):
            S1 = mpool.tile([128, 4, 32, 32], F32R, tag="S1")
            nc.sync.dma_start(out=S1.rearrange("p a b c -> p (a b c)"),
                              in_=xs1_d[j, :, :])
            map1 = mpool.tile([128, 4, 34, 34], F32R, tag="map1")
            map2 = mpool.tile([128, 4, 18, 18], F32R, tag="map2")
            map3 = mpool.tile([128, 8, 18, 18], F32R, tag="map3")
            map4 = mpool.tile([128, 8, 10, 10], F32R, tag="map4")
            map5 = m5pool.tile([128, 16, 10, 10], F32R, tag="map5")
            for m, nb, Hp in ((map1, 4, 34), (map2, 4, 18), (map3, 8, 18),
                              (map4, 8, 10), (map5, 16, 10)):
                zv = zline[:, :nb * Hp].rearrange("p (a b) -> p a b", a=nb)
                nc.vector.tensor_copy(m[:, :, 0, :], zv)
                nc.vector.tensor_copy(m[:, :, Hp - 1, :], zv)
                nc.vector.tensor_copy(m[:, :, :, 0], zv)
                nc.vector.tensor_copy(m[:, :, :, Hp - 1], zv)

            # ---- conv1: K=27 im2col, 4 diagonal (sub) tile-MMs ----
            for t in range(8):
                b, yh = t // 2, (t % 2) * 16
                ps = cps.tile([128, 512], F32, tag="cps")
                nc.tensor.matmul(ps, wc1, S1[:, b, yh:yh + 16, :],
                                 start=True, stop=True)
                sc, bi = bscale(0)
                nc.scalar.activation(map1[:, b, 1 + yh:17 + yh, 1:33],
                                     ps.rearrange("q (y x) -> q y x", y=16),
                                     Relu, bias=bi, scale=sc)

            # ---- conv2: 9 block-diag shift MMs ----
            for t in range(8):
                b, yh = t // 2, (t % 2) * 16
                ps = cps.tile([128, 512], F32, tag="cps")
                for i, (dy, dx) in enumerate(SHIFTS):
                    nc.tensor.matmul(
                        ps, wc[("wc2", i)],
                        map1[:, b, dy + yh:dy + yh + 16, dx:dx + 32],
                        start=(i == 0), stop=(i == 8))
                tmp = tpool.tile([128, 16, 32], F32, tag="ev2")
                sc, bi = bscale(1)
                nc.scalar.activation(tmp, ps.rearrange("q (y x) -> q y x", y=16),
                                     Relu, bias=bi, scale=sc)
                t1 = tpool.tile([128, 16, 16], F32, tag="p2a")
                v = tmp.rearrange("p y (xo two) -> p y xo two", two=2)
                nc.vector.tensor_max(t1, v[:, :, :, 0], v[:, :, :, 1])
                v2 = t1.rearrange("p (yo two) x -> p yo two x", two=2)
                yo = (t % 2) * 8
                nc.vector.tensor_max(map2[:, b, 1 + yo:9 + yo, 1:17],
                                     v2[:, :, 0, :], v2[:, :, 1, :])

            # ---- conv3: 32->64, 2 passes x 2 diag tiles ----
            for t in range(2):
                for p in range(2):
                    ps = cps.tile([128, 512], F32, tag="cps")
                    for i, (dy, dx) in enumerate(SHIFTS):
                        nc.tensor.matmul(
                            ps, wc[("wc3", p, i)],
                            map2[:, 2 * t:2 * t + 2, dy:dy + 16, dx:dx + 16],
                            start=(i == 0), stop=(i == 8))
                    psv = ps.rearrange("q (b y x) -> q b y x", b=2, y=16)
                    for s2 in range(2):
                        sc, bi = bscale(2, 64 * s2, 64)
                        dst = map3[64 * s2:64 * s2 + 64, 4 * t + p:4 * t + p + 3:2,
                                   1:17, 1:17]
                        nc.scalar.activation(dst, psv[64 * s2:64 * s2 + 64],
                                             Relu, bias=bi, scale=sc)

            # ---- conv4: 64->64, diag 2 tiles, pooled ----
            for t in range(4):
                ps = cps.tile([128, 512], F32, tag="cps")
                for i, (dy, dx) in enumerate(SHIFTS):
                    nc.tensor.matmul(
                        ps, wc[("wc4", i)],
                        map3[:, 2 * t:2 * t + 2, dy:dy + 16, dx:dx + 16],
                        start=(i == 0), stop=(i == 8))
                tmp = tpool.tile([128, 2, 16, 16], F32, tag="ev4")
                sc, bi = bscale(3)
                nc.scalar.activation(tmp, ps.rearrange("q (b y x) -> q b y x", b=2, y=16),
                                     Relu, bias=bi, scale=sc)
                t1 = tpool.tile([128, 2, 16, 8], F32, tag="p4a")
                v = tmp.rearrange("p b y (xo two) -> p b y xo two", two=2)
                nc.vector.tensor_max(t1, v[:, :, :, :, 0], v[:, :, :, :, 1])
                v2 = t1.rearrange("p b (yo two) x -> p b yo two x", two=2)
                nc.vector.tensor_max(map4[:, 2 * t:2 * t + 2, 1:9, 1:9],
                                     v2[:, :, :, 0, :], v2[:, :, :, 1, :])

            # ---- conv5: 64->128, 2 passes ----
            for p in range(2):
                ps = cps.tile([128, 512], F32, tag="cps")
                for i, (dy, dx) in enumerate(SHIFTS):
                    nc.tensor.matmul(
                        ps, wc[("wc5", p, i)],
                        map4[:, :, dy:dy + 8, dx:dx + 8],
                        start=(i == 0), stop=(i == 8))
                sc, bi = bscale(4)
                nc.scalar.activation(
                    map5[:, p:16:2, 1:9, 1:9],
                    ps.rearrange("q (b y x) -> q b y x", b=8, y=8),
                    Relu, bias=bi, scale=sc)

            # ---- conv6: 128->128, pooled -> f ----
            for t in range(2):
                ps = cps.tile([128, 512], F32, tag="cps")
                for i, (dy, dx) in enumerate(SHIFTS):
                    nc.tensor.matmul(
                        ps, wc[("wc6", i)],
                        map5[:, 8 * t:8 * t + 8, dy:dy + 8, dx:dx + 8],
                        start=(i == 0), stop=(i == 8))
                tmp = tpool.tile([128, 8, 8, 8], F32, tag="ev6")
                sc, bi = bscale(5)
                nc.scalar.activation(tmp, ps.rearrange("q (b y x) -> q b y x", b=8, y=8),
                                     Relu, bias=bi, scale=sc)
                t1 = tpool.tile([128, 8, 8, 4], F32, tag="p6a")
                v = tmp.rearrange("p b y (xo two) -> p b y xo two", two=2)
                nc.vector.tensor_max(t1, v[:, :, :, :, 0], v[:, :, :, :, 1])
                ft = tpool.tile([128, 8, 16], F32, tag="p6b")
                nc.vector.tensor_max(ft.rearrange("p b (yo x) -> p b yo x", yo=4),
                                     t1[:, :, 0:8:2, :], t1[:, :, 1:8:2, :])
                ns = CH * j + 8 * t
                nc.scalar.copy(f_r[:, ns:ns + 8, :], ft)
                nc.vector.tensor_copy(f_b[:, ns:ns + 8, :], ft)
            if DEBUG and j == 0:
                for nm, tl in (("map1", map1), ("map2", map2), ("map3", map3),
                               ("map4", map4), ("map5", map5)):
                    nc.sync.dma_start(out=dbg[nm][:, :, :, :], in_=tl[:, :, :, :])

        # ================= gate =================
        gps = eps.tile([128, 128], F32, tag="eps")
        for p in range(16):
            nc.tensor.matmul(gps, wg1[p], f_r[:, :, p],
                             start=(p == 0), stop=(p == 15))
        hg = hpool.tile([128, 128], F32R, tag="hg")
        nc.scalar.activation(hg, gps, Relu, bias=gst[:, 1:2], scale=gst[:, 0:1])
        lps = tps.tile([8, 128], F32, tag="tps")
        nc.tensor.matmul(lps, wg2, hg, start=True, stop=True)
        logits = spool.tile([8, 128], F32, tag="logits")
        nc.scalar.activation(logits, lps, Ident, bias=gb2[:, 0:1], scale=1.0)

        # ================= experts (bf16) =================
        eo = []
        for e in range(8):
            h1 = []
            for h in range(2):
                ps1 = eps.tile([128, 128], F32, tag="eps")
                for p in range(16):
                    wt = epool.tile([128, 128], BF16, tag="we1")
                    nc.sync.dma_start(out=wt, in_=we1_d[e, p, :, 128 * h:128 * h + 128])
                    nc.tensor.matmul(ps1, wt, f_b[:, :, p],
                                     start=(p == 0), stop=(p == 15))
                h1t = hpool.tile([128, 128], BF16, tag=f"h1_{h}")
                nc.scalar.activation(h1t, ps1, Relu,
                                     bias=et1[:, 2 * e + h:2 * e + h + 1],
                                     scale=es1[:, 2 * e + h:2 * e + h + 1])
                h1.append(h1t)
            h2 = []
            for oh in range(2):
                ps2 = eps.tile([128, 128], F32, tag="eps")
                for kh in range(2):
                    wt = epool.tile([128, 128], BF16, tag="we2")
                    nc.sync.dma_start(out=wt, in_=we2_d[e, kh, :, 128 * oh:128 * oh + 128])
                    nc.tensor.matmul(ps2, wt, h1[kh],
                                     start=(kh == 0), stop=(kh == 1))
                h2t = hpool.tile([128, 128], BF16, tag=f"h2_{oh}")
                nc.scalar.activation(h2t, ps2, Relu,
                                     bias=et2[:, 2 * e + oh:2 * e + oh + 1],
                                     scale=es2[:, 2 * e + oh:2 * e + oh + 1])
                h2.append(h2t)
            ps3 = tps.tile([10, 128], F32, tag="tps")
            for kh in range(2):
                wt = epool.tile([128, 10], BF16, tag="we3")
                nc.sync.dma_start(out=wt, in_=we3_d[e, kh, :, :])
                nc.tensor.matmul(ps3, wt, h2[kh], start=(kh == 0), stop=(kh == 1))
            eot = spool.tile([10, 128], F32, tag=f"eo_{e}")
            nc.scalar.activation(eot, ps3, Ident, bias=eb3[:, e:e + 1], scale=1.0)
            eo.append(eot)

        # ================= top-k gates + combine =================
        ltp = tps.tile([128, 8], F32, tag="tps")
        nc.tensor.transpose(ltp, logits, id8[:, :])
        v = spool.tile([128, 8], F32, tag="v")
        nc.scalar.copy(v, ltp)
        mx1 = spool.tile([128, 1], F32, tag="mx1")
        nc.vector.tensor_reduce(out=mx1, in_=v, axis=mybir.AxisListType.X, op=OP.max)
        msk = spool.tile([128, 8], F32, tag="msk")
        nc.vector.tensor_scalar(out=msk, in0=v, scalar1=mx1, scalar2=-1e30,
                                op0=OP.is_equal, op1=OP.mult)
        v2 = spool.tile([128, 8], F32, tag="v2")
        nc.vector.tensor_add(v2, v, msk)
        mx2 = spool.tile([128, 1], F32, tag="mx2")
        nc.vector.tensor_reduce(out=mx2, in_=v2, axis=mybir.AxisListType.X, op=OP.max)
        d = spool.tile([128, 1], F32, tag="d")
        nc.vector.tensor_sub(d, mx2, mx1)
        ex = spool.tile([128, 1], F32, tag="ex")
        nc.scalar.activation(ex, d, Exp)
        den = spool.tile([128, 1], F32, tag="den")
        nc.scalar.add(den, ex, 1.0)
        g1 = spool.tile([128, 1], F32, tag="g1")
        nc.vector.reciprocal(g1, den)
        g2 = spool.tile([128, 1], F32, tag="g2")
        nc.vector.tensor_mul(g2, ex, g1)
        ga = spool.tile([128, 8], F32, tag="ga")
        nc.vector.tensor_scalar(out=ga, in0=v, scalar1=mx1, scalar2=g1,
                                op0=OP.is_equal, op1=OP.mult)
        gb = spool.tile([128, 8], F32, tag="gb")
        nc.vector.tensor_scalar(out=gb, in0=v2, scalar1=mx2, scalar2=g2,
                                op0=OP.is_equal, op1=OP.mult)
        gates = spool.tile([128, 8], F32, tag="gates")
        nc.vector.tensor_add(gates, ga, gb)

        if DEBUG:
            nc.sync.dma_start(out=dbg["f"][:, :, :], in_=f_r[:, :, :])
            nc.sync.dma_start(out=dbg["logits"][:, :], in_=logits[:, :])
            nc.sync.dma_start(out=dbg["gates"][:, :], in_=gates[:, :])
            nc.sync.dma_start(out=dbg["eo0"][:, :], in_=eo[0][:, :])
        acc = hpool.tile([128, 10], F32, tag="acc")
        nc.vector.memset(acc, 0.0)
        for e in range(8):
            pt = tps.tile([128, 10], F32, tag="tps")
            nc.tensor.transpose(pt, eo[e], id10[:, :])
            tm = hpool.tile([128, 10], F32, tag="tm")
            nc.vector.tensor_scalar(out=tm, in0=pt, scalar1=gates[:, e:e + 1],
                                    scalar2=None, op0=OP.mult)
            acc2 = hpool.tile([128, 10], F32, tag="acc")
            nc.vector.tensor_add(acc2, acc, tm)
            acc = acc2
        nc.sync.dma_start(out=out_d[:, :], in_=acc)

        for p in (tps, eps, cps, spool, hpool, epool, m5pool, fpool, tpool, mpool, wpool):
            p.release()


def kernel(x, conv_params, expert_params, gate_params):
    global LAST_RESULTS
    shared, xs1_cores = _prep_host(x, conv_params, expert_params, gate_params)

    nc = bacc.Bacc("TRN2", target_bir_lowering=False, debug=False)
    _build(nc)
    nc.compile()

    in_maps = []
    for k in range(N_CORES):
        m = dict(shared)
        m["xs1"] = xs1_cores[k]
        in_maps.append(m)
    global LAST_NC, LAST_IN_MAPS
    LAST_NC = nc
    LAST_IN_MAPS = in_maps
    LAST_RESULTS = run_bass_kernel_spmd(nc, in_maps, core_ids=list(range(N_CORES)))
    out = np.concatenate([r["out"] for r in LAST_RESULTS.results], axis=0)
    return out.astype(np.float32)
